# revision 1
# baseline (speedup 1.0000x reference)
"""Multi-head causal self-attention (B=2, S=2048, D=2048, H=16) on 8 TRN2 cores.

Sharding: data parallel on batch (2) x tensor parallel on head groups (4 heads
per core). Each core computes QKV projections for its 512 q/k/v channels, the
causal attention for its 4 heads, and a partial output projection against its
512 columns of Wo. The host sums the 4 partials per batch and adds bo.

All matmul operands are fp16 (full PE rate, fast weight load, fp22 multiply
with fp32 accumulate); softmax statistics stay fp32. Scores are computed
directly in [k, q] orientation so the exp'd tiles feed the PV matmul as the
moving operand with no transposes; row sums come from an all-ones stationary
matmul (replicated across partitions) and normalization happens on the
PSUM->SBUF copy.
"""

import math
from contextlib import ExitStack

import numpy as np

import concourse.bass as bass
import concourse.tile as tile
from concourse import bacc, mybir
from concourse.bass_utils import run_bass_kernel_spmd

B, S, D, H, HD = 2, 2048, 2048, 16, 128
N_CORES = 8
HPC = 4          # heads per core
HJ = HPC * HD    # 512 projection channels per core
SG = 512         # column-group width for matmuls
ND = D // 128    # 16 contraction tiles over model dim
NS = S // 128    # 16 tiles over sequence
NG = S // SG     # 4 column groups over sequence

F32 = mybir.dt.float32
F16 = mybir.dt.float16
AX = mybir.AxisListType.X
ADD = mybir.AluOpType.add
MUL = mybir.AluOpType.mult
EXP = mybir.ActivationFunctionType.Exp

last_exec_time_ns = None


def _build():
    nc = bacc.Bacc("TRN2", target_bir_lowering=False, debug=False)

    xt = nc.dram_tensor("xt", [D, S], F16, kind="ExternalInput").ap()
    wq = nc.dram_tensor("wq", [D, HJ], F16, kind="ExternalInput").ap()
    wk = nc.dram_tensor("wk", [D, HJ], F16, kind="ExternalInput").ap()
    wv = nc.dram_tensor("wv", [D, HJ], F16, kind="ExternalInput").ap()
    wo = nc.dram_tensor("wo", [HJ, D], F16, kind="ExternalInput").ap()
    bq = nc.dram_tensor("bq", [HJ, 1], F32, kind="ExternalInput").ap()
    bk = nc.dram_tensor("bk", [HJ, 1], F32, kind="ExternalInput").ap()
    bv = nc.dram_tensor("bv", [1, HJ], F16, kind="ExternalInput").ap()
    mask = nc.dram_tensor("mask", [128, 128], F32, kind="ExternalInput").ap()
    ones = nc.dram_tensor("ones", [1, 128], F16, kind="ExternalInput").ap()
    out = nc.dram_tensor("out", [S, D], F32, kind="ExternalOutput").ap()

    with tile.TileContext(nc) as tc, ExitStack() as es:
        cpool = es.enter_context(tc.tile_pool(name="const", bufs=1))
        mask_sb = cpool.tile([128, 128], F32, name="mask", tag="mask")
        nc.sync.dma_start(mask_sb[:], mask[:])
        ones_sb = cpool.tile([1, 128], F16, name="ones", tag="ones")
        nc.sync.dma_start(ones_sb[:], ones[:])
        bv_sb = cpool.tile([1, HJ], F16, name="bv", tag="bv")
        nc.sync.dma_start(bv_sb[:], bv[:])
        onesm_sb = cpool.tile([128, 128], F16, name="onesm_sb", tag="onesm")
        nc.gpsimd.memset(onesm_sb[:], 1.0)
        bq_sb = []
        bk_sb = []
        for i in range(HPC):
            t = cpool.tile([128, 1], F32, name=f"bq{i}", tag=f"bq{i}")
            nc.sync.dma_start(t[:], bq[i * 128:(i + 1) * 128, :])
            bq_sb.append(t)
            t = cpool.tile([128, 1], F32, name=f"bk{i}", tag=f"bk{i}")
            nc.sync.dma_start(t[:], bk[i * 128:(i + 1) * 128, :])
            bk_sb.append(t)

        rpool = es.enter_context(tc.tile_pool(name="res", bufs=1))
        qT = [rpool.tile([128, S], F16, name=f"qT{i}", tag=f"qT{i}")
              for i in range(HPC)]
        kT = [rpool.tile([128, S], F16, name=f"kT{i}", tag=f"kT{i}")
              for i in range(HPC)]
        vsb = [rpool.tile([128, HJ], F16, name=f"v{j}", tag=f"v{j}")
               for j in range(NS)]

        # ---------------- phase 1: q/k/v projections ----------------------
        # qT[h]/kT[h] = W_h @ x^T via lhsT = W^T tiles (stationary) over
        # contraction d, rhs = x^T column groups. v in natural [s, hj] layout
        # via stationary x^T slices, moving Wv^T. Weight DMAs are batched as
        # [128, 512] tiles covering all 4 heads, loaded lazily inside the
        # first column group so the sync queue never starves the PE.
        with tc.tile_pool(name="wqk", bufs=1) as wpool, \
             tc.tile_pool(name="xt1", bufs=8) as xpool, \
             tc.tile_pool(name="wvp", bufs=1) as wvpool, \
             tc.tile_pool(name="xtv", bufs=8) as vxpool, \
             tc.tile_pool(name="ps1", bufs=1, space="PSUM") as ps1:
            wtile = {}
            wvt = {}
            for sg in range(NG):
                if sg == NG - 2:
                    # prefetch the v-projection weights while qk still computes
                    for d in range(ND):
                        t = wvpool.tile([128, HJ], F16, name=f"wv{d}",
                                        tag=f"wv{d}")
                        nc.sync.dma_start(t[:], wv[d * 128:(d + 1) * 128, :])
                        wvt[d] = t
                ps = {}
                for i in range(HPC):
                    ps[("q", i)] = ps1.tile([128, SG], F32, name=f"psa{i}",
                                            tag=f"a{i}")
                    ps[("k", i)] = ps1.tile([128, SG], F32, name=f"psb{i}",
                                            tag=f"b{i}")
                for d in range(ND):
                    xtile = xpool.tile([128, SG], F16, name="xtile", tag="xt")
                    nc.sync.dma_start(
                        xtile[:], xt[d * 128:(d + 1) * 128,
                                     sg * SG:(sg + 1) * SG])
                    for which, wdram in (("q", wq), ("k", wk)):
                        if (which, d) not in wtile:
                            t = wpool.tile([128, SG], F16, name=f"w{which}{d}",
                                           tag=f"w{which}{d}")
                            nc.sync.dma_start(
                                t[:], wdram[d * 128:(d + 1) * 128, :])
                            wtile[(which, d)] = t
                        for i in range(HPC):
                            nc.tensor.matmul(
                                ps[(which, i)][:],
                                lhsT=wtile[(which, d)][:, i * 128:(i + 1) * 128],
                                rhs=xtile[:],
                                start=(d == 0), stop=(d == ND - 1))
                for i in range(HPC):
                    nc.vector.tensor_scalar_add(
                        qT[i][:, sg * SG:(sg + 1) * SG], ps[("q", i)][:],
                        bq_sb[i][:])
                    nc.vector.tensor_scalar_add(
                        kT[i][:, sg * SG:(sg + 1) * SG], ps[("k", i)][:],
                        bk_sb[i][:])

            # v pass (re-streams x^T through its own pool; psum banks reuse
            # the q/k tags, alternating by sg parity for cross-sg overlap)
            for sg in range(NG):
                ab = "a" if sg % 2 == 0 else "b"
                ps = [ps1.tile([128, HJ], F32, name=f"psv{i}", tag=f"{ab}{i}")
                      for i in range(4)]
                for d in range(ND):
                    xtile = vxpool.tile([128, SG], F16, name="xtile", tag="xt")
                    nc.sync.dma_start(
                        xtile[:], xt[d * 128:(d + 1) * 128,
                                     sg * SG:(sg + 1) * SG])
                    for ss in range(4):
                        nc.tensor.matmul(
                            ps[ss][:],
                            lhsT=xtile[:, ss * 128:(ss + 1) * 128],
                            rhs=wvt[d][:],
                            start=(d == 0), stop=False)
                for ss in range(4):
                    # bias: rank-1 ones (x) bv accumulated into the same group
                    nc.tensor.matmul(
                        ps[ss][:], lhsT=ones_sb[:],
                        rhs=bv_sb[:], start=False, stop=True)
                    nc.vector.tensor_copy(vsb[sg * 4 + ss][:], ps[ss][:])

        # ---------------- phases 2+3: attention + output projection -------
        with tc.tile_pool(name="attn", bufs=1) as apool, \
             tc.tile_pool(name="wo", bufs=1) as wopool:
            attn = [apool.tile([128, S], F16, name=f"at{h}", tag=f"at{h}")
                    for h in range(HPC)]
            wot = []
            for t_ in range(HPC):
                wt = wopool.tile([128, D], F16, name=f"wo{t_}", tag=f"wo{t_}")
                nc.gpsimd.dma_start(wt[:], wo[t_ * 128:(t_ + 1) * 128, :])
                wot.append(wt)

            # phase 2: scores in [k, q] orientation; exp'd tiles feed PV as
            # the moving operand; sums via all-ones stationary (replicated
            # across partitions); normalize on the PSUM->SBUF copy. Units
            # ordered g-descending; phase 3 shares the pv psum slots and
            # backfills PE gaps as attn columns complete (st descending).
            with tc.tile_pool(name="et", bufs=8) as etpool, \
                 tc.tile_pool(name="sm", bufs=6) as spool, \
                 tc.tile_pool(name="ost", bufs=3) as opool, \
                 tc.tile_pool(name="ps_sc", bufs=3, space="PSUM") as ps_sc, \
                 tc.tile_pool(name="ps_x", bufs=2, space="PSUM") as ps_x, \
                 tc.tile_pool(name="ps_pv", bufs=3, space="PSUM") as ps_pv:
                for g in range(NG - 1, -1, -1):
                    nkt = 4 * g + 4
                    for h in range(HPC):
                        po = ps_pv.tile([128, SG], F32, name="popv", tag="pv")
                        sm = ps_x.tile([128, SG], F32, name="smps", tag="x")
                        for kt in range(nkt):
                            jlo = max(0, kt - 4 * g)
                            qoff = jlo * 128
                            w = SG - qoff
                            psc = ps_sc.tile([128, SG], F32, name="psc",
                                             tag="sc")
                            nc.tensor.matmul(
                                psc[:, :w],
                                lhsT=kT[h][:, kt * 128:(kt + 1) * 128],
                                rhs=qT[h][:, g * SG + qoff:(g + 1) * SG],
                                start=True, stop=True)
                            if kt >= 4 * g:
                                # diagonal block is this tile's first 128 cols
                                nc.vector.tensor_tensor(
                                    psc[:, 0:128], psc[:, 0:128],
                                    mask_sb[:], op=ADD)
                            et = etpool.tile([128, SG], F16, name="et",
                                             tag="et")
                            nc.scalar.activation(et[:, :w], psc[:, :w], EXP)
                            nc.tensor.matmul(
                                po[:, qoff:],
                                lhsT=vsb[kt][:, h * 128:(h + 1) * 128],
                                rhs=et[:, :w],
                                start=(kt == 0), stop=(kt == nkt - 1))
                            nc.tensor.matmul(
                                sm[:, qoff:],
                                lhsT=onesm_sb[:],
                                rhs=et[:, :w],
                                start=(kt == 0), stop=(kt == nkt - 1))
                        rr = spool.tile([128, SG], F32, name="rr", tag="rr")
                        nc.vector.reciprocal(rr[:], sm[:])
                        nc.vector.tensor_tensor(
                            attn[h][:, g * SG:(g + 1) * SG], po[:], rr[:],
                            op=MUL)

                    # phase 3 slice for this g level: output rows st = 4g..4g+3
                    for st in range(4 * g + 3, 4 * g - 1, -1):
                        for dg in range(NG):
                            po3 = ps_pv.tile([128, SG], F32, name="po3",
                                             tag="pv")
                            for h in range(HPC):
                                nc.tensor.matmul(
                                    po3[:],
                                    lhsT=attn[h][:, st * 128:(st + 1) * 128],
                                    rhs=wot[h][:, dg * SG:(dg + 1) * SG],
                                    start=(h == 0), stop=(h == HPC - 1))
                            ot = opool.tile([128, SG], F32, name="ost",
                                            tag="ost")
                            nc.vector.tensor_copy(ot[:], po3[:])
                            nc.gpsimd.dma_start(
                                out[st * 128:(st + 1) * 128,
                                    dg * SG:(dg + 1) * SG], ot[:])

    nc.finalize()
    return nc


_NC_CACHE = []


def kernel(hidden_states, Wq, bq, Wk, bk, Wv, bv, Wo, bo, **_unused):
    global last_exec_time_ns

    hidden_states = np.asarray(hidden_states, dtype=np.float32)
    Wq = np.asarray(Wq, dtype=np.float32)
    Wk = np.asarray(Wk, dtype=np.float32)
    Wv = np.asarray(Wv, dtype=np.float32)
    Wo = np.asarray(Wo, dtype=np.float32)
    bq = np.asarray(bq, dtype=np.float32)
    bk = np.asarray(bk, dtype=np.float32)
    bv = np.asarray(bv, dtype=np.float32)
    bo = np.asarray(bo, dtype=np.float32)

    if not _NC_CACHE:
        _NC_CACHE.append(_build())
    nc = _NC_CACHE[0]

    scale = 1.0 / math.sqrt(HD)
    q_idx = np.arange(128)[:, None]
    k_idx = np.arange(128)[None, :]
    # [k, q] orientation: keep k <= q
    mask = np.where(k_idx.T <= q_idx.T, 0.0, -50.0).astype(np.float32)
    ones = np.ones((1, 128), np.float16)

    xts = [np.ascontiguousarray(hidden_states[b].T).astype(np.float16)
           for b in range(B)]
    in_maps = []
    for c in range(N_CORES):
        b, hg = divmod(c, HPC)
        sl = slice(hg * HJ, (hg + 1) * HJ)
        in_maps.append({
            "xt": xts[b],
            "wq": np.ascontiguousarray((Wq[sl] * scale).T).astype(np.float16),
            "wk": np.ascontiguousarray(Wk[sl].T).astype(np.float16),
            "wv": np.ascontiguousarray(Wv[sl].T).astype(np.float16),
            "wo": np.ascontiguousarray(Wo[:, sl].T).astype(np.float16),
            "bq": (bq[sl] * scale).reshape(HJ, 1).copy(),
            "bk": bk[sl].reshape(HJ, 1).copy(),
            "bv": bv[sl].reshape(1, HJ).astype(np.float16),
            "mask": mask,
            "ones": ones,
        })

    res = run_bass_kernel_spmd(nc, in_maps, core_ids=list(range(N_CORES)))
    last_exec_time_ns = res.exec_time_ns

    outp = np.empty((B, S, D), np.float32)
    for b in range(B):
        acc = res.results[b * HPC]["out"].astype(np.float32)
        for c in range(b * HPC + 1, (b + 1) * HPC):
            acc = acc + res.results[c]["out"]
        outp[b] = acc + bo[None, :]
    return outp

